# revision 21
# baseline (speedup 1.0000x reference)
"""AUGRU (DIEN attention layer) on 8 Trainium2 NeuronCores via Bass/Tile.

Problem: B=2048, T=200, D=128, H=128 fp32 AUGRU scan with per-row sequence
lengths (zero output + state carry past seq_len).

Strategy (pure batch data parallelism, 256 rows per core):
  - Transposed layout [feature=128 partitions, batch free]; weights stationary.
  - All-bf16 datapath (fp32 accumulation in PSUM; tolerance is 2e-2).
  - Rows are globally sorted by seq_len (descending) and dealt round-robin to
    cores, so every core sees the same alive-row profile.  At step t only the
    first W_t columns are still alive; all instruction widths shrink with t
    (~44% of column-work skipped for uniform seq_len).
  - h_t = wv_t + v_t is accumulated into the next step's gate matmuls as
    wv@W + v@W (linearity), so only v (ready right after tanh) sits on the
    critical chain; hn materializes off-chain for the elementwise consumers.
  - Critical chain per step: tanh -> v=au*c -> mm(v@Whr) -> sigmoid(r) ->
    rh=r*h -> mm(rh@Whc) -> tanh.  sigmoid(u), au, q, wv, x-projections,
    wv-side matmuls and the output DMA all run off-chain.
  - am = att*mask(t<seq_len) premultiplied on host: state freezes past
    seq_len with no masking work on device; outputs past seq_len are zeroed
    on the host during unsharding.
"""

import os

import numpy as np
import ml_dtypes

import concourse.bacc as bacc
import concourse.mybir as mybir
import concourse.tile as tile
from concourse.bass_utils import run_bass_kernel_spmd

F32 = mybir.dt.float32
BF16 = mybir.dt.bfloat16
AF = mybir.ActivationFunctionType
OP = mybir.AluOpType

B, T, D, H = 2048, 200, 128, 128
NCORES = 8
BL = B // NCORES  # 256 batch rows per core
TB = 25           # timesteps per input DMA block

LAST_EXEC_TIME_NS = None
_NC_CACHE = {}


def _build_kernel(bg_const, bc_const, widths):
    nsteps = len(widths)
    nc = bacc.Bacc("TRN2", target_bir_lowering=False, debug=False, num_devices=NCORES)

    xT = nc.dram_tensor("xT", [128, T * BL], BF16, kind="ExternalInput")
    am = nc.dram_tensor("am", [128, T * BL], BF16, kind="ExternalInput")
    wnames = ["wxr", "whr", "wxu", "whu", "wxc", "whc"]
    wd = {n: nc.dram_tensor(n, [128, 128], BF16, kind="ExternalInput") for n in wnames}
    bgr = nc.dram_tensor("bgr", [128, 1], F32, kind="ExternalInput")
    bgu = nc.dram_tensor("bgu", [128, 1], F32, kind="ExternalInput")
    bcv = nc.dram_tensor("bcv", [128, 1], F32, kind="ExternalInput")
    outT = nc.dram_tensor("outT", [128, T, BL], BF16, kind="ExternalOutput")

    with tile.TileContext(nc) as tc:
        with (
            tc.tile_pool(name="w", bufs=1) as wpool,
            tc.tile_pool(name="xb", bufs=3) as xpool,
            tc.tile_pool(name="ab", bufs=3) as apool,
            tc.tile_pool(name="h", bufs=4) as hpool,
            tc.tile_pool(name="s", bufs=3) as spool,
            tc.tile_pool(name="ps", bufs=2, space="PSUM") as ppool,
        ):
            w = {}
            for n in wnames:
                wt = wpool.tile([128, 128], BF16, tag=n, name=f"w_{n}")
                nc.sync.dma_start(wt[:], wd[n].ap())
                w[n] = wt
            btiles = {}
            if bg_const is None:
                for n, dt_ in (("bgr", bgr), ("bgu", bgu)):
                    bt = wpool.tile([128, 1], F32, tag=n, name=f"b_{n}")
                    nc.sync.dma_start(bt[:], dt_.ap())
                    btiles[n] = bt
            if bc_const is None:
                bt = wpool.tile([128, 1], F32, tag="bcv", name="b_bcv")
                nc.sync.dma_start(bt[:], bcv.ap())
                btiles["bcv"] = bt
            bias_r = bg_const if bg_const is not None else btiles["bgr"][:]
            bias_u = bg_const if bg_const is not None else btiles["bgu"][:]
            bias_c = bc_const if bc_const is not None else btiles["bcv"][:]

            mm = nc.tensor.matmul
            tt = nc.vector.tensor_tensor

            def load_block(blk):
                lo, hi = blk * TB * BL, (blk + 1) * TB * BL
                xbt = xpool.tile([128, TB * BL], BF16, tag="xb", name=f"xb_{blk}")
                nc.sync.dma_start(xbt[:], xT.ap()[:, lo:hi])
                abt = apool.tile([128, TB * BL], BF16, tag="ab", name=f"ab_{blk}")
                nc.sync.dma_start(abt[:], am.ap()[:, lo:hi])
                return xbt, abt

            nblocks = (len(widths) + TB - 1) // TB
            # ---- t = 0 (h0 = 0: r-path dead, h1 = (am*sigmoid(zu_x))*tanh(zc_x))
            W0 = widths[0]
            xb, ab = load_block(0)
            nxt = load_block(1) if nblocks > 1 else None

            pu = ppool.tile([128, 256], F32, tag="pu", name="pu_0")
            pc = ppool.tile([128, 256], F32, tag="pc", name="pc_0")
            mm(pu[:, 0:W0], w["wxu"][:], xb[:, 0:W0], start=True, stop=True)
            mm(pc[:, 0:W0], w["wxc"][:], xb[:, 0:W0], start=True, stop=True)
            u0 = spool.tile([128, BL], BF16, tag="u", name="u_0")
            nc.scalar.activation(u0[:, 0:W0], pu[:, 0:W0], AF.Sigmoid, bias=bias_u)
            c0 = spool.tile([128, BL], BF16, tag="c", name="c_0")
            nc.scalar.activation(c0[:, 0:W0], pc[:, 0:W0], AF.Tanh, bias=bias_c)
            au0 = spool.tile([128, BL], BF16, tag="au", name="au_0")
            tt(au0[:, 0:W0], ab[:, 0:W0], u0[:, 0:W0], OP.mult)
            h = hpool.tile([128, BL], BF16, tag="h", name="h_0")
            tt(h[:, 0:W0], au0[:, 0:W0], c0[:, 0:W0], OP.mult)
            nc.sync.dma_start(outT.ap()[:, 0, 0:W0], h[:, 0:W0])

            wv_prev = None   # wv_{t-1}/v_{t-1} (None at t=1: h0 is not split)
            v_prev = None
            pw = 0           # width at which wv_prev/v_prev were produced
            for t in range(1, nsteps):
                W = widths[t]
                blk, tl = divmod(t, TB)
                if tl == 0:
                    # switch to the prefetched block; prefetch the one after
                    xb, ab = nxt
                    nxt = load_block(blk + 1) if blk + 1 < nblocks else None
                off = tl * BL
                xt = xb[:, off:off + W]
                amt = ab[:, off:off + W]

                pr = ppool.tile([128, 256], F32, tag="pr", name=f"pr_{t}")
                pu = ppool.tile([128, 256], F32, tag="pu", name=f"pu_{t}")
                pc = ppool.tile([128, 256], F32, tag="pc", name=f"pc_{t}")
                # x-projections first: no h dependency, keeps PE busy early.
                mm(pr[:, 0:W], w["wxr"][:], xt, start=True, stop=False)
                mm(pu[:, 0:W], w["wxu"][:], xt, start=True, stop=False)
                mm(pc[:, 0:W], w["wxc"][:], xt, start=True, stop=False)
                if wv_prev is None:
                    mm(pr[:, 0:W], w["whr"][:], h[:, 0:W], start=False, stop=True)
                else:
                    # pr is on the chain: split h = wv + v so its stop-part
                    # (v, ready right after tanh) runs in PE slot 2.  pu is
                    # off-chain: one matmul on the materialized hn keeps the
                    # PE queue in readiness order (scheduler-proof).
                    mm(pr[:, 0:W], w["whr"][:], wv_prev[:, 0:W],
                       start=False, stop=False)
                    mm(pr[:, 0:W], w["whr"][:], v_prev[:, 0:W],
                       start=False, stop=True)
                mm(pu[:, 0:W], w["whu"][:], h[:, 0:W], start=False, stop=True)

                r = spool.tile([128, BL], BF16, tag="r", name=f"r_{t}")
                nc.scalar.activation(r[:, 0:W], pr[:, 0:W], AF.Sigmoid, bias=bias_r)
                u = spool.tile([128, BL], BF16, tag="u", name=f"u_{t}")
                nc.scalar.activation(u[:, 0:W], pu[:, 0:W], AF.Sigmoid, bias=bias_u)

                rh = spool.tile([128, BL], BF16, tag="rh", name=f"rh_{t}")
                tt(rh[:, 0:W], r[:, 0:W], h[:, 0:W], OP.mult)
                au = spool.tile([128, BL], BF16, tag="au", name=f"au_{t}")
                tt(au[:, 0:W], amt, u[:, 0:W], OP.mult)
                q = spool.tile([128, BL], BF16, tag="q", name=f"q_{t}")
                tt(q[:, 0:W], au[:, 0:W], h[:, 0:W], OP.mult)
                wv = spool.tile([128, BL], BF16, tag="wv", name=f"wv_{t}")
                tt(wv[:, 0:W], h[:, 0:W], q[:, 0:W], OP.subtract)
                mm(pc[:, 0:W], w["whc"][:], rh[:, 0:W], start=False, stop=True)

                cc = spool.tile([128, BL], BF16, tag="c", name=f"c_{t}")
                nc.scalar.activation(cc[:, 0:W], pc[:, 0:W], AF.Tanh, bias=bias_c)
                v = spool.tile([128, BL], BF16, tag="v", name=f"v_{t}")
                tt(v[:, 0:W], au[:, 0:W], cc[:, 0:W], OP.mult)
                hn = hpool.tile([128, BL], BF16, tag="h", name=f"h_{t}")
                tt(hn[:, 0:W], wv[:, 0:W], v[:, 0:W], OP.add)
                h = hn
                wv_prev, v_prev, pw = wv, v, W
                nc.sync.dma_start(outT.ap()[:, t, 0:W], hn[:, 0:W])
    nc.compile()
    return nc


def _prep_inputs(inputs, att_scores, seq_len, Wg, bg, Wc, bc):
    x = np.asarray(inputs, dtype=np.float32)
    att = np.asarray(att_scores, dtype=np.float32)
    sl = np.asarray(seq_len, dtype=np.int32)
    Wg = np.asarray(Wg, dtype=np.float32)
    bg = np.asarray(bg, dtype=np.float32)
    Wc = np.asarray(Wc, dtype=np.float32)
    bc = np.asarray(bc, dtype=np.float32)

    # Sort rows by seq_len descending, deal round-robin to cores: every core
    # gets the same alive-count profile and columns are packed so the live
    # rows at step t are a prefix.
    order = np.argsort(-sl, kind="stable")
    nsteps = max(1, int(sl.max()))
    a_t = (np.arange(nsteps)[None, :] < sl[order][:, None]).sum(0)  # alive rows
    per_core = -(-a_t // NCORES)                                    # ceil
    widths = np.minimum(BL, np.maximum(8, ((per_core + 7) // 8) * 8))
    widths = [int(v) for v in widths]

    m = (np.arange(T, dtype=np.int32)[None, :] < sl[:, None])
    am = (att * m).astype(ml_dtypes.bfloat16)

    bg_const = float(bg.flat[0]) if np.all(bg == bg.flat[0]) else None
    bc_const = float(bc.flat[0]) if np.all(bc == bc.flat[0]) else None

    wmats = {
        "wxr": Wg[0:128, 0:128], "whr": Wg[128:256, 0:128],
        "wxu": Wg[0:128, 128:256], "whu": Wg[128:256, 128:256],
        "wxc": Wc[0:128, :], "whc": Wc[128:256, :],
    }
    wmats = {k: np.ascontiguousarray(v.astype(ml_dtypes.bfloat16))
             for k, v in wmats.items()}
    bgr = np.ascontiguousarray(bg[0:128, None])
    bgu = np.ascontiguousarray(bg[128:256, None])
    bcv = np.ascontiguousarray(bc[:, None])

    x16 = x.astype(ml_dtypes.bfloat16)
    in_maps = []
    rows_per_core = []
    for k in range(NCORES):
        rows = order[k::NCORES]
        rows_per_core.append(rows)
        xk = np.ascontiguousarray(x16[rows].transpose(2, 1, 0))     # [D, T, BL]
        amk = np.ascontiguousarray(
            np.broadcast_to(am[rows].T[None, :, :], (128, T, BL)))  # [128, T, BL]
        in_maps.append({
            "xT": xk.reshape(128, T * BL),
            "am": amk.reshape(128, T * BL),
            **wmats,
            "bgr": bgr, "bgu": bgu, "bcv": bcv,
        })
    return in_maps, bg_const, bc_const, m, widths, rows_per_core


def kernel(inputs, att_scores, seq_len, Wg, bg, Wc, bc):
    global LAST_EXEC_TIME_NS
    in_maps, bg_const, bc_const, mask, widths, rows_per_core = _prep_inputs(
        inputs, att_scores, seq_len, Wg, bg, Wc, bc)

    key = (bg_const, bc_const, tuple(widths))
    if key not in _NC_CACHE:
        _NC_CACHE[key] = _build_kernel(bg_const, bc_const, widths)
    nc = _NC_CACHE[key]

    trace = bool(int(os.environ.get("AUGRU_TRACE", "0")))
    kwargs = {}
    if trace:
        kwargs["trace"] = True
        tmpdir = os.environ.get("AUGRU_TRACE_DIR")
        if tmpdir:
            os.makedirs(tmpdir, exist_ok=True)
            kwargs["tmpdir"] = tmpdir
    try:
        res = run_bass_kernel_spmd(nc, in_maps, list(range(NCORES)), **kwargs)
    except Exception:
        if not kwargs:
            raise
        # profiling is best-effort; retry without it
        res = run_bass_kernel_spmd(nc, in_maps, list(range(NCORES)))
    LAST_EXEC_TIME_NS = res.exec_time_ns

    out = np.empty((B, T, H), np.float32)
    for k in range(NCORES):
        o = res.results[k]["outT"]                        # [128, T, BL] bf16
        out[rows_per_core[k]] = o.astype(np.float32).transpose(2, 1, 0)
    # kill never-written / past-seq_len columns (may be uninitialized memory)
    out = np.where(mask[:, :, None], out, np.float32(0.0))
    return out
